# revision 28
# baseline (speedup 1.0000x reference)
"""Trainium2 Bass kernel for CLIPAttention (B=32, S=512, E=768, H=12, D=64).

Strategy: data-parallel over batch across 8 NeuronCores (4 batches/core).
All matmul operands fp16 (PSUM accumulates fp32). The kernel is PE-issue
bound, so the design minimizes tensor-engine instructions:

  - hidden_states is transposed to feature-major ON THE HOST and DMA'd
    straight into SBUF: no on-chip transposes at all.
  - q,k projections feature-major (lhsT = W tile, rhs = xt, N=512).
  - v token-major, written into v_ext[:, i, h, 0:64] with cols 64:128 of
    each head slot holding ones: the PV matmul lhsT = [v_h | ones] then
    computes BOTH the PV product (PSUM rows 0:64) and the softmax
    denominator replicated on rows 64:128 - the separate ones-matmul and
    the gpsimd partition-broadcast of the baseline are gone.
  - scores computed k-major (scoresT = kh.T @ qh) triangular; exp on ACT
    straight to fp16 SBUF; diagonal block masked by a 0/1 triu multiply.
  - out projection FEATURE-major (lhsT = Wo tile, rhs = outT), output is
    DMA'd feature-major and transposed back on the host.
  - emission is software-pipelined across batches: scores/PV of batch b
    interleave with the projections of batch b+1 so the PE never waits
    on the ACT exp chain.
"""

import os
import time

import numpy as np
from contextlib import ExitStack

import concourse.bass as bass
import concourse.mybir as mybir
import concourse.tile as tile
from concourse import bacc
from concourse.bass_utils import run_bass_kernel_spmd
from concourse.masks import make_upper_triangular

B, S, E, H, D = 32, 512, 768, 12, 64
NCORES = 8
NB = B // NCORES          # batches per core
P = 128
KT = E // P               # 6 feature tiles
QT = S // P               # 4 token tiles
SCALE = float(D) ** -0.5  # 0.125
F32 = mybir.dt.float32
F16 = mybir.dt.float16

AF = mybir.ActivationFunctionType
OP = mybir.AluOpType


def _build():
    nc = bacc.Bacc(trn_type="TRN2")

    hsT = nc.dram_tensor("hsT", [NB, E, S], F16, kind="ExternalInput")
    w_dr = {}
    b_dr = {}
    for nm in ("q", "k", "v", "o"):
        w_dr[nm] = nc.dram_tensor(f"W{nm}", [E, E], F16, kind="ExternalInput")
        b_dr[nm] = nc.dram_tensor(f"b{nm}", [E], F32, kind="ExternalInput")
    out = nc.dram_tensor("out_fm", [NB, E, S], F32, kind="ExternalOutput")

    with ExitStack() as ctx:
        tc = ctx.enter_context(tile.TileContext(nc))

        singles = ctx.enter_context(tc.tile_pool(name="singles", bufs=1))
        xtpool = ctx.enter_context(tc.tile_pool(name="xtpool", bufs=2))
        qkpool = ctx.enter_context(tc.tile_pool(name="qkpool", bufs=2))
        pepool = ctx.enter_context(tc.tile_pool(name="pepool", bufs=6))
        rpool = ctx.enter_context(tc.tile_pool(name="rpool", bufs=3))
        otpool = ctx.enter_context(tc.tile_pool(name="otpool", bufs=2))
        opool = ctx.enter_context(tc.tile_pool(name="opool", bufs=3))

        ps_mm = ctx.enter_context(tc.tile_pool(name="ps_mm", bufs=3, space="PSUM"))
        ps_s = ctx.enter_context(tc.tile_pool(name="ps_s", bufs=4, space="PSUM"))
        ps_pv = ctx.enter_context(tc.tile_pool(name="ps_pv", bufs=1, space="PSUM"))

        # ---- constants ----
        # upper-triangular (incl diagonal) 0/1 mask: keeps q >= k entries of
        # a k-major diagonal block
        triu01 = singles.tile([P, P], F16, name="triu01")
        make_upper_triangular(nc, triu01, val=1.0, diag=True)

        # weights (pre-cast fp16 on host); per-k-tile DMAs so the first
        # projection can start as soon as its tiles land, and the transfers
        # spread across DMA queues
        w_sb = {}
        for nm in ("q", "k", "v", "o"):
            w_sb[nm] = singles.tile([P, KT, E], F16, name=f"w_{nm}")
        xt_tiles = {}

        def dma_w(nm):
            nc.sync.dma_start(
                out=w_sb[nm], in_=w_dr[nm].rearrange("(ko p) m -> p ko m", p=P)
            )

        def dma_xt(b):
            # issue x transfers from the (idle) gpsimd engine: a second DGE
            # ring, so x flows in parallel with the weight DMAs instead of
            # queuing behind them
            nc.gpsimd.dma_start(
                out=xt_tiles[b], in_=hsT[b].rearrange("(ko p) t -> p ko t", p=P)
            )

        for b in range(NB):
            xt_tiles[b] = xtpool.tile([P, KT, S], F16, name=f"xt_{b}", tag="xt")

        # bulk transfers in first-needed order; ONE dma_start per tensor
        # (descriptor issue on the sync engine is ~0.7us each, so fewer,
        # bigger issues start the data flowing sooner - the single striped
        # hardware queue gives full bandwidth either way)
        # first piece: just the m=0 column block of Wq plus batch-0 x, so
        # the very first projection chunk starts after ~1MB of DMA
        nc.sync.dma_start(
            out=w_sb["q"][:, :, :P],
            in_=w_dr["q"].rearrange("(ko p) m -> p ko m", p=P)[:, :, :P],
        )
        dma_xt(0)
        nc.sync.dma_start(
            out=w_sb["q"][:, :, P:],
            in_=w_dr["q"].rearrange("(ko p) m -> p ko m", p=P)[:, :, P:],
        )
        # biases are tiny but gate PSUM evacuation; right after the first two
        dma_w("k")
        bias_pp = {}
        for nm in ("q", "k", "o"):
            bias_pp[nm] = singles.tile([P, KT], F32, name=f"bpp_{nm}")
            nc.sync.dma_start(
                out=bias_pp[nm], in_=b_dr[nm].rearrange("(ko p) -> p ko", p=P)
            )
        # broadcast-to-all-partitions bias form for token-major v
        bias_bc = {}
        for nm in ("v",):
            bias_bc[nm] = singles.tile([P, E], F32, name=f"bbc_{nm}")
            src = b_dr[nm][:]
            bcast = bass.AP(tensor=src.tensor, offset=src.offset, ap=[[0, P], *src.ap])
            nc.sync.dma_start(out=bias_bc[nm], in_=bcast)
        dma_w("v")
        dma_xt(1)
        dma_w("o")
        dma_xt(2)
        dma_xt(3)

        # v_ext: [P, QT, H, 128] fp16; per head slot cols 0:64 hold ones,
        # cols 64:128 hold v_h (so the PV matmul also produces the softmax
        # denominator on PSUM partitions 0:64, where the custom-DVE
        # reciprocal can read it - it requires partition base 0). Two manual
        # buffers with the ones columns initialized once each.
        v_ext_bufs = []
        for i in range(2):
            vb = singles.tile([P, QT, H, P], F16, name=f"v_ext{i}")
            nc.vector.memset(vb[:, :, :, :D], 1.0)
            v_ext_bufs.append(vb)

        # ---------- per-batch state ----------
        qk_of = {}     # b -> {"q": tile, "k": tile}
        outT_of = {}   # b -> tile
        pE_live = {}   # (b,h) -> tile

        # ---------- chunk emitters ----------
        def qk_chunk(b, nm, m):
            if m == 0 and nm == "q":
                qk_of[b] = {}
            if nm not in qk_of[b]:
                qk_of[b][nm] = qkpool.tile(
                    [P, KT, S], F16, name=f"{nm}T_{b}", tag=f"{nm}T"
                )
            dst = qk_of[b][nm]
            xt = xt_tiles[b]
            ps = ps_mm.tile([P, S], F32, name=f"ps{nm}_{b}_{m}", tag="mm")
            for kk in range(KT):
                nc.tensor.matmul(
                    ps,
                    lhsT=w_sb[nm][:, kk, m * P:(m + 1) * P],
                    rhs=xt[:, kk, :],
                    start=(kk == 0),
                    stop=(kk == KT - 1),
                )
            if m % 2 == 0 and b < 3:
                nc.scalar.activation(
                    out=dst[:, m, :],
                    in_=ps,
                    func=AF.Identity,
                    bias=bias_pp[nm][:, m:m + 1],
                    scale=1.0,
                )
            else:
                nc.vector.tensor_scalar_add(
                    out=dst[:, m, :],
                    in0=ps,
                    scalar1=bias_pp[nm][:, m:m + 1],
                )

        def v_chunk(b, i):
            v_ext = v_ext_bufs[b % 2]
            xt = xt_tiles[b]
            # kk-outer so both N-chunks share one stationary load per k-tile
            pss = [
                ps_mm.tile([P, S], F32, name=f"psv_{b}_{i}_{n}", tag="mm")
                for n in range(2)
            ]
            for kk in range(KT):
                for n in range(2):
                    mm = nc.tensor.matmul(
                        pss[n][:, :384],
                        lhsT=xt[:, kk, i * P:(i + 1) * P],
                        rhs=w_sb["v"][:, kk, n * 384:(n + 1) * 384],
                        start=(kk == 0),
                        stop=(kk == KT - 1),
                    )
                    if n == 1:
                        mm.ldweights = False
            for n in range(2):
                nc.vector.tensor_tensor(
                    out=v_ext[:, i, n * 6:(n + 1) * 6, D:],
                    in0=pss[n][:, :384].rearrange("p (h c) -> p h c", c=D),
                    in1=bias_bc["v"][:, n * 384:(n + 1) * 384].rearrange(
                        "p (h c) -> p h c", c=D
                    ),
                    op=OP.add,
                )

        def sc_pair_chunk(b, h0, js):
            """Scores for heads h0 (even, PE rows 0:64 -> tile (0,0)) and
            h0+1 (odd, PE rows 64:128 -> tile (64,0)), j-interleaved so the
            two row-tiles of the 64x128-mode array compute concurrently."""
            heads = (h0, h0 + 1)
            if js[0] == 0:
                for h in heads:
                    pE_live[(b, h)] = pepool.tile(
                        [P, QT, S], F16, name=f"pE_{b}_{h}", tag="pE"
                    )
            pEs = {h: pE_live[(b, h)] for h in heads}
            for j in js:
                q0 = j * P
                n_mm = S - q0
                for h in heads:
                    g, rr = h // 2, h % 2
                    qh = qk_of[b]["q"][rr * D:(rr + 1) * D, g, :]
                    kh = qk_of[b]["k"][rr * D:(rr + 1) * D, g, :]
                    pE = pEs[h]
                    ps = ps_s.tile([P, S], F32, name=f"pss_{b}_{h}_{j}", tag="s")
                    nc.tensor.matmul(
                        ps[:, :n_mm],
                        lhsT=kh[:, j * P:(j + 1) * P],
                        rhs=qh[:, q0:],
                        start=True,
                        stop=True,
                    )
                    nc.scalar.activation(
                        out=pE[:, j, q0:],
                        in_=ps[:, :n_mm],
                        func=AF.Exp,
                        scale=SCALE,
                    )
                    # causal mask on the diagonal block: keep q >= k (on the
                    # otherwise-idle gpsimd engine, off the DVE critical path)
                    nc.gpsimd.tensor_tensor(
                        out=pE[:, j, q0:q0 + P],
                        in0=pE[:, j, q0:q0 + P],
                        in1=triu01,
                        op=OP.mult,
                    )

        def pv_chunk(b, h):
            g, rr = h // 2, h % 2
            if h == 0:
                outT_of[b] = otpool.tile(
                    [P, KT, S], F16, name=f"outT_{b}", tag="outT"
                )
            v_ext = v_ext_bufs[b % 2]
            pE = pE_live.pop((b, h))
            # PV + denominator in one matmul: lhsT = [v_h | ones]
            po = ps_pv.tile([P, S], F32, name=f"po_{b}_{h}", tag="pv")
            for j in range(QT):
                nc.tensor.matmul(
                    po[:, j * P:],
                    lhsT=v_ext[:, j, h, :],
                    rhs=pE[:, j, j * P:],
                    start=(j == 0),
                    stop=(j == QT - 1),
                    skip_group_check=True,
                )
            # reciprocal of the denominator (replicated on rows 0:64)
            rden = rpool.tile([D, S], F32, name=f"rden_{b}_{h}", tag="rden")
            nc.vector.reciprocal_approx_fast(rden, po[:D, :])
            # normalization fused into the PSUM->SBUF copy; partition-
            # shifted write puts odd heads at partitions 64:127 directly
            nc.vector.tensor_tensor(
                out=outT_of[b][rr * D:(rr + 1) * D, g, :],
                in0=po[D:, :],
                in1=rden,
                op=OP.mult,
            )

        def o_chunk(b, m):
            outT = outT_of[b]
            ps = ps_mm.tile([P, S], F32, name=f"pso_{b}_{m}", tag="mm")
            for kk in range(KT):
                nc.tensor.matmul(
                    ps,
                    lhsT=w_sb["o"][:, kk, m * P:(m + 1) * P],
                    rhs=outT[:, kk, :],
                    start=(kk == 0),
                    stop=(kk == KT - 1),
                )
            o_sb = opool.tile([P, S], F32, name=f"o_{b}_{m}", tag="o")
            if m % 2 == 0 and b < 2:
                nc.scalar.activation(
                    out=o_sb,
                    in_=ps,
                    func=AF.Identity,
                    bias=bias_pp["o"][:, m:m + 1],
                    scale=1.0,
                )
            else:
                nc.vector.tensor_scalar_add(
                    out=o_sb, in0=ps, scalar1=bias_pp["o"][:, m:m + 1]
                )
            nc.sync.dma_start(out=out[b, m * P:(m + 1) * P, :], in_=o_sb)

        # ---------- software-pipelined emission ----------
        def emit_slot(d_batch, w_chunks):
            """Interleave scores/pv of batch d_batch with the given list of
            projection chunk thunks (next batch's q/k/v, prior batch's o)."""
            if d_batch is None:
                for w in w_chunks:
                    w()
                return
            wi = 0

            def take_w(k):
                nonlocal wi
                for _ in range(k):
                    if wi < len(w_chunks):
                        w_chunks[wi]()
                        wi += 1

            for p in range(H // 2):
                sc_pair_chunk(d_batch, 2 * p, (0, 1))
                take_w(1)
                sc_pair_chunk(d_batch, 2 * p, (2,))
                if p >= 1:
                    pv_chunk(d_batch, 2 * p - 2)
                sc_pair_chunk(d_batch, 2 * p, (3,))
                take_w(1)
                if p >= 1:
                    pv_chunk(d_batch, 2 * p - 1)
                take_w(1)
            pv_chunk(d_batch, H - 2)
            take_w(2)
            pv_chunk(d_batch, H - 1)
            take_w(len(w_chunks) - wi)

        def qk_chunks(b):
            return [
                (lambda b=b, nm=nm, m=m: qk_chunk(b, nm, m))
                for nm in ("q", "k")
                for m in range(KT)
            ]

        def v_chunks(b):
            return [(lambda b=b, i=i: v_chunk(b, i)) for i in range(QT)]

        def o_chunks(b):
            return [(lambda b=b, m=m: o_chunk(b, m)) for m in range(KT)]

        # slot 0: projections of batch 0 only
        emit_slot(None, qk_chunks(0) + v_chunks(0))
        # steady-state slots; v(3) runs at the head of slot 4 so the thin
        # tail slot still has PE work to hide the batch-3 exp chain
        emit_slot(0, qk_chunks(1) + v_chunks(1))
        emit_slot(1, qk_chunks(2) + v_chunks(2) + o_chunks(0))
        emit_slot(2, qk_chunks(3) + o_chunks(1))
        emit_slot(3, v_chunks(3) + o_chunks(2))
        emit_slot(None, o_chunks(3))

    nc.compile()
    return nc


_NC_CACHE = None


def _get_nc():
    global _NC_CACHE
    if _NC_CACHE is None:
        _NC_CACHE = _build()
    return _NC_CACHE


def run(inputs, trace=False):
    if trace:
        os.environ.pop("BASS_NEVER_TRACE", None)
    else:
        # keep the spmd runner off the NTFF trace path (the profiling hook
        # module is not always present)
        os.environ["BASS_NEVER_TRACE"] = "1"
    # host prep: cast to fp16 and pre-transpose hidden_states to
    # feature-major (identical rounding to an on-chip cast, and it removes
    # every on-chip transpose)
    hs = np.asarray(inputs["hidden_states"], dtype=np.float32).astype(np.float16)
    assert hs.shape == (B, S, E)
    hsT = np.ascontiguousarray(hs.transpose(0, 2, 1))  # [B, E, S]
    wb = {}
    for nm in ("q", "k", "v", "o"):
        wb[f"W{nm}"] = np.ascontiguousarray(
            np.asarray(inputs[f"W{nm}"], dtype=np.float32).astype(np.float16)
        )
        wb[f"b{nm}"] = np.ascontiguousarray(
            np.asarray(inputs[f"b{nm}"], dtype=np.float32)
        )

    nc = _get_nc()
    in_maps = []
    for c in range(NCORES):
        m = {"hsT": hsT[c * NB:(c + 1) * NB]}
        m.update(wb)
        in_maps.append(m)
    res = run_bass_kernel_spmd(
        nc, in_maps, core_ids=list(range(NCORES)), trace=trace
    )
    # gather: outputs are feature-major [NB, E, S]; transpose back on host
    out_fm = np.concatenate([r_["out_fm"] for r_ in res.results], axis=0)
    outp = np.ascontiguousarray(out_fm.transpose(0, 2, 1))
    return outp, res


def kernel(**inputs) -> np.ndarray:
    # retry once on transient accelerator errors (rare NRT exec glitches)
    last = None
    for attempt in range(2):
        try:
            outp, _ = run(inputs, trace=False)
            return outp
        except Exception as e:  # noqa: BLE001
            last = e
            time.sleep(10)
    raise last


# revision 29
# speedup vs baseline: 1.2106x; 1.2106x over previous
"""Trainium2 Bass kernel for CLIPAttention (B=32, S=512, E=768, H=12, D=64).

Strategy: data-parallel over batch across 8 NeuronCores (4 batches/core).
All matmul operands fp16 (PSUM accumulates fp32). The kernel is PE-issue
bound, so the design minimizes tensor-engine instructions:

  - hidden_states is transposed to feature-major ON THE HOST and DMA'd
    straight into SBUF: no on-chip transposes at all.
  - q,k projections feature-major (lhsT = W tile, rhs = xt, N=512).
  - v token-major, written into v_ext[:, i, h, 0:64] with cols 64:128 of
    each head slot holding ones: the PV matmul lhsT = [v_h | ones] then
    computes BOTH the PV product (PSUM rows 0:64) and the softmax
    denominator replicated on rows 64:128 - the separate ones-matmul and
    the gpsimd partition-broadcast of the baseline are gone.
  - scores computed k-major (scoresT = kh.T @ qh) triangular; exp on ACT
    straight to fp16 SBUF; diagonal block masked by a 0/1 triu multiply.
  - out projection FEATURE-major (lhsT = Wo tile, rhs = outT), output is
    DMA'd feature-major and transposed back on the host.
  - emission is software-pipelined across batches: scores/PV of batch b
    interleave with the projections of batch b+1 so the PE never waits
    on the ACT exp chain.
"""

import os
import time

import numpy as np
from contextlib import ExitStack

import concourse.bass as bass
import concourse.mybir as mybir
import concourse.tile as tile
from concourse import bacc
from concourse.bass_utils import run_bass_kernel_spmd
from concourse.masks import make_upper_triangular

B, S, E, H, D = 32, 512, 768, 12, 64
NCORES = 8
NB = B // NCORES          # batches per core
P = 128
KT = E // P               # 6 feature tiles
QT = S // P               # 4 token tiles
SCALE = float(D) ** -0.5  # 0.125
F32 = mybir.dt.float32
F16 = mybir.dt.float16

AF = mybir.ActivationFunctionType
OP = mybir.AluOpType


def _build():
    nc = bacc.Bacc(trn_type="TRN2")

    hsT = nc.dram_tensor("hsT", [NB, E, S], F16, kind="ExternalInput")
    w_dr = {}
    b_dr = {}
    for nm in ("q", "k", "v", "o"):
        w_dr[nm] = nc.dram_tensor(f"W{nm}", [E, E], F16, kind="ExternalInput")
        b_dr[nm] = nc.dram_tensor(f"b{nm}", [E], F32, kind="ExternalInput")
    out = nc.dram_tensor("out_fm", [NB, E, S], F32, kind="ExternalOutput")

    with ExitStack() as ctx:
        tc = ctx.enter_context(tile.TileContext(nc))

        singles = ctx.enter_context(tc.tile_pool(name="singles", bufs=1))
        xtpool = ctx.enter_context(tc.tile_pool(name="xtpool", bufs=2))
        qkpool = ctx.enter_context(tc.tile_pool(name="qkpool", bufs=2))
        pepool = ctx.enter_context(tc.tile_pool(name="pepool", bufs=6))
        rpool = ctx.enter_context(tc.tile_pool(name="rpool", bufs=3))
        otpool = ctx.enter_context(tc.tile_pool(name="otpool", bufs=2))
        opool = ctx.enter_context(tc.tile_pool(name="opool", bufs=3))

        ps_mm = ctx.enter_context(tc.tile_pool(name="ps_mm", bufs=3, space="PSUM"))
        ps_s = ctx.enter_context(tc.tile_pool(name="ps_s", bufs=4, space="PSUM"))
        ps_pv = ctx.enter_context(tc.tile_pool(name="ps_pv", bufs=1, space="PSUM"))

        # ---- constants ----
        # upper-triangular (incl diagonal) 0/1 mask: keeps q >= k entries of
        # a k-major diagonal block
        triu01 = singles.tile([P, P], F16, name="triu01")
        make_upper_triangular(nc, triu01, val=1.0, diag=True)

        # weights (pre-cast fp16 on host); per-k-tile DMAs so the first
        # projection can start as soon as its tiles land, and the transfers
        # spread across DMA queues
        w_sb = {}
        for nm in ("q", "k", "v", "o"):
            w_sb[nm] = singles.tile([P, KT, E], F16, name=f"w_{nm}")
        xt_tiles = {}

        def dma_w(nm):
            nc.sync.dma_start(
                out=w_sb[nm], in_=w_dr[nm].rearrange("(ko p) m -> p ko m", p=P)
            )

        def dma_xt(b):
            nc.sync.dma_start(
                out=xt_tiles[b], in_=hsT[b].rearrange("(ko p) t -> p ko t", p=P)
            )

        for b in range(NB):
            xt_tiles[b] = xtpool.tile([P, KT, S], F16, name=f"xt_{b}", tag="xt")

        # bulk transfers in first-needed order; ONE dma_start per tensor
        # (descriptor issue on the sync engine is ~0.7us each, so fewer,
        # bigger issues start the data flowing sooner - the single striped
        # hardware queue gives full bandwidth either way)
        # first piece: just the m=0 column block of Wq plus batch-0 x, so
        # the very first projection chunk starts after ~1MB of DMA
        nc.sync.dma_start(
            out=w_sb["q"][:, :, :P],
            in_=w_dr["q"].rearrange("(ko p) m -> p ko m", p=P)[:, :, :P],
        )
        dma_xt(0)
        nc.sync.dma_start(
            out=w_sb["q"][:, :, P:],
            in_=w_dr["q"].rearrange("(ko p) m -> p ko m", p=P)[:, :, P:],
        )
        # biases are tiny but gate PSUM evacuation; right after the first two
        dma_w("k")
        bias_pp = {}
        for nm in ("q", "k", "o"):
            bias_pp[nm] = singles.tile([P, KT], F32, name=f"bpp_{nm}")
            nc.sync.dma_start(
                out=bias_pp[nm], in_=b_dr[nm].rearrange("(ko p) -> p ko", p=P)
            )
        # broadcast-to-all-partitions bias form for token-major v
        bias_bc = {}
        for nm in ("v",):
            bias_bc[nm] = singles.tile([P, E], F32, name=f"bbc_{nm}")
            src = b_dr[nm][:]
            bcast = bass.AP(tensor=src.tensor, offset=src.offset, ap=[[0, P], *src.ap])
            nc.sync.dma_start(out=bias_bc[nm], in_=bcast)
        dma_w("v")
        dma_xt(1)
        dma_w("o")
        dma_xt(2)
        dma_xt(3)

        # v_ext: [P, QT, H, 128] fp16; per head slot cols 0:64 hold ones,
        # cols 64:128 hold v_h (so the PV matmul also produces the softmax
        # denominator on PSUM partitions 0:64, where the custom-DVE
        # reciprocal can read it - it requires partition base 0). Two manual
        # buffers with the ones columns initialized once each.
        v_ext_bufs = []
        for i in range(2):
            vb = singles.tile([P, QT, H, P], F16, name=f"v_ext{i}")
            nc.vector.memset(vb[:, :, :, :D], 1.0)
            v_ext_bufs.append(vb)

        # ---------- per-batch state ----------
        qk_of = {}     # b -> {"q": tile, "k": tile}
        outT_of = {}   # b -> tile
        pE_live = {}   # (b,h) -> tile

        # ---------- chunk emitters ----------
        def qk_chunk(b, nm, m):
            if m == 0 and nm == "q":
                qk_of[b] = {}
            if nm not in qk_of[b]:
                qk_of[b][nm] = qkpool.tile(
                    [P, KT, S], F16, name=f"{nm}T_{b}", tag=f"{nm}T"
                )
            dst = qk_of[b][nm]
            xt = xt_tiles[b]
            ps = ps_mm.tile([P, S], F32, name=f"ps{nm}_{b}_{m}", tag="mm")
            for kk in range(KT):
                nc.tensor.matmul(
                    ps,
                    lhsT=w_sb[nm][:, kk, m * P:(m + 1) * P],
                    rhs=xt[:, kk, :],
                    start=(kk == 0),
                    stop=(kk == KT - 1),
                )
            if m % 2 == 0 and b < 3:
                nc.scalar.activation(
                    out=dst[:, m, :],
                    in_=ps,
                    func=AF.Identity,
                    bias=bias_pp[nm][:, m:m + 1],
                    scale=1.0,
                )
            else:
                nc.vector.tensor_scalar_add(
                    out=dst[:, m, :],
                    in0=ps,
                    scalar1=bias_pp[nm][:, m:m + 1],
                )

        def v_chunk(b, i):
            v_ext = v_ext_bufs[b % 2]
            xt = xt_tiles[b]
            # kk-outer so both N-chunks share one stationary load per k-tile
            pss = [
                ps_mm.tile([P, S], F32, name=f"psv_{b}_{i}_{n}", tag="mm")
                for n in range(2)
            ]
            for kk in range(KT):
                for n in range(2):
                    mm = nc.tensor.matmul(
                        pss[n][:, :384],
                        lhsT=xt[:, kk, i * P:(i + 1) * P],
                        rhs=w_sb["v"][:, kk, n * 384:(n + 1) * 384],
                        start=(kk == 0),
                        stop=(kk == KT - 1),
                    )
                    if n == 1:
                        mm.ldweights = False
            for n in range(2):
                nc.vector.tensor_tensor(
                    out=v_ext[:, i, n * 6:(n + 1) * 6, D:],
                    in0=pss[n][:, :384].rearrange("p (h c) -> p h c", c=D),
                    in1=bias_bc["v"][:, n * 384:(n + 1) * 384].rearrange(
                        "p (h c) -> p h c", c=D
                    ),
                    op=OP.add,
                )

        def sc_pair_chunk(b, h0, js):
            """Scores for heads h0 (even, PE rows 0:64 -> tile (0,0)) and
            h0+1 (odd, PE rows 64:128 -> tile (64,0)), j-interleaved so the
            two row-tiles of the 64x128-mode array compute concurrently."""
            heads = (h0, h0 + 1)
            if js[0] == 0:
                for h in heads:
                    pE_live[(b, h)] = pepool.tile(
                        [P, QT, S], F16, name=f"pE_{b}_{h}", tag="pE"
                    )
            pEs = {h: pE_live[(b, h)] for h in heads}
            for j in js:
                q0 = j * P
                n_mm = S - q0
                for h in heads:
                    g, rr = h // 2, h % 2
                    qh = qk_of[b]["q"][rr * D:(rr + 1) * D, g, :]
                    kh = qk_of[b]["k"][rr * D:(rr + 1) * D, g, :]
                    pE = pEs[h]
                    ps = ps_s.tile([P, S], F32, name=f"pss_{b}_{h}_{j}", tag="s")
                    nc.tensor.matmul(
                        ps[:, :n_mm],
                        lhsT=kh[:, j * P:(j + 1) * P],
                        rhs=qh[:, q0:],
                        start=True,
                        stop=True,
                    )
                    nc.scalar.activation(
                        out=pE[:, j, q0:],
                        in_=ps[:, :n_mm],
                        func=AF.Exp,
                        scale=SCALE,
                    )
                    # causal mask on the diagonal block: keep q >= k (on the
                    # otherwise-idle gpsimd engine, off the DVE critical path)
                    nc.gpsimd.tensor_tensor(
                        out=pE[:, j, q0:q0 + P],
                        in0=pE[:, j, q0:q0 + P],
                        in1=triu01,
                        op=OP.mult,
                    )

        def pv_chunk(b, h):
            g, rr = h // 2, h % 2
            if h == 0:
                outT_of[b] = otpool.tile(
                    [P, KT, S], F16, name=f"outT_{b}", tag="outT"
                )
            v_ext = v_ext_bufs[b % 2]
            pE = pE_live.pop((b, h))
            # PV + denominator in one matmul: lhsT = [v_h | ones]
            po = ps_pv.tile([P, S], F32, name=f"po_{b}_{h}", tag="pv")
            for j in range(QT):
                nc.tensor.matmul(
                    po[:, j * P:],
                    lhsT=v_ext[:, j, h, :],
                    rhs=pE[:, j, j * P:],
                    start=(j == 0),
                    stop=(j == QT - 1),
                    skip_group_check=True,
                )
            # reciprocal of the denominator (replicated on rows 0:64)
            rden = rpool.tile([D, S], F32, name=f"rden_{b}_{h}", tag="rden")
            nc.vector.reciprocal_approx_fast(rden, po[:D, :])
            # normalization fused into the PSUM->SBUF copy; partition-
            # shifted write puts odd heads at partitions 64:127 directly
            nc.vector.tensor_tensor(
                out=outT_of[b][rr * D:(rr + 1) * D, g, :],
                in0=po[D:, :],
                in1=rden,
                op=OP.mult,
            )

        def o_chunk(b, m):
            outT = outT_of[b]
            ps = ps_mm.tile([P, S], F32, name=f"pso_{b}_{m}", tag="mm")
            for kk in range(KT):
                nc.tensor.matmul(
                    ps,
                    lhsT=w_sb["o"][:, kk, m * P:(m + 1) * P],
                    rhs=outT[:, kk, :],
                    start=(kk == 0),
                    stop=(kk == KT - 1),
                )
            o_sb = opool.tile([P, S], F32, name=f"o_{b}_{m}", tag="o")
            if m % 2 == 0 and b < 2:
                nc.scalar.activation(
                    out=o_sb,
                    in_=ps,
                    func=AF.Identity,
                    bias=bias_pp["o"][:, m:m + 1],
                    scale=1.0,
                )
            else:
                nc.vector.tensor_scalar_add(
                    out=o_sb, in0=ps, scalar1=bias_pp["o"][:, m:m + 1]
                )
            nc.sync.dma_start(out=out[b, m * P:(m + 1) * P, :], in_=o_sb)

        # ---------- software-pipelined emission ----------
        def emit_slot(d_batch, w_chunks):
            """Interleave scores/pv of batch d_batch with the given list of
            projection chunk thunks (next batch's q/k/v, prior batch's o)."""
            if d_batch is None:
                for w in w_chunks:
                    w()
                return
            wi = 0

            def take_w(k):
                nonlocal wi
                for _ in range(k):
                    if wi < len(w_chunks):
                        w_chunks[wi]()
                        wi += 1

            for p in range(H // 2):
                sc_pair_chunk(d_batch, 2 * p, (0, 1))
                take_w(1)
                sc_pair_chunk(d_batch, 2 * p, (2,))
                if p >= 1:
                    pv_chunk(d_batch, 2 * p - 2)
                sc_pair_chunk(d_batch, 2 * p, (3,))
                take_w(1)
                if p >= 1:
                    pv_chunk(d_batch, 2 * p - 1)
                take_w(1)
            pv_chunk(d_batch, H - 2)
            take_w(2)
            pv_chunk(d_batch, H - 1)
            take_w(len(w_chunks) - wi)

        def qk_chunks(b):
            return [
                (lambda b=b, nm=nm, m=m: qk_chunk(b, nm, m))
                for nm in ("q", "k")
                for m in range(KT)
            ]

        def v_chunks(b):
            return [(lambda b=b, i=i: v_chunk(b, i)) for i in range(QT)]

        def o_chunks(b):
            return [(lambda b=b, m=m: o_chunk(b, m)) for m in range(KT)]

        # slot 0: projections of batch 0 only
        emit_slot(None, qk_chunks(0) + v_chunks(0))
        # steady-state slots; v(3) runs at the head of slot 4 so the thin
        # tail slot still has PE work to hide the batch-3 exp chain
        emit_slot(0, qk_chunks(1) + v_chunks(1))
        emit_slot(1, qk_chunks(2) + v_chunks(2) + o_chunks(0))
        emit_slot(2, qk_chunks(3) + o_chunks(1))
        emit_slot(3, v_chunks(3) + o_chunks(2))
        emit_slot(None, o_chunks(3))

    nc.compile()
    return nc


_NC_CACHE = None


def _get_nc():
    global _NC_CACHE
    if _NC_CACHE is None:
        _NC_CACHE = _build()
    return _NC_CACHE


def run(inputs, trace=False):
    if trace:
        os.environ.pop("BASS_NEVER_TRACE", None)
    else:
        # keep the spmd runner off the NTFF trace path (the profiling hook
        # module is not always present)
        os.environ["BASS_NEVER_TRACE"] = "1"
    # host prep: cast to fp16 and pre-transpose hidden_states to
    # feature-major (identical rounding to an on-chip cast, and it removes
    # every on-chip transpose)
    hs = np.asarray(inputs["hidden_states"], dtype=np.float32).astype(np.float16)
    assert hs.shape == (B, S, E)
    hsT = np.ascontiguousarray(hs.transpose(0, 2, 1))  # [B, E, S]
    wb = {}
    for nm in ("q", "k", "v", "o"):
        wb[f"W{nm}"] = np.ascontiguousarray(
            np.asarray(inputs[f"W{nm}"], dtype=np.float32).astype(np.float16)
        )
        wb[f"b{nm}"] = np.ascontiguousarray(
            np.asarray(inputs[f"b{nm}"], dtype=np.float32)
        )

    nc = _get_nc()
    in_maps = []
    for c in range(NCORES):
        m = {"hsT": hsT[c * NB:(c + 1) * NB]}
        m.update(wb)
        in_maps.append(m)
    res = run_bass_kernel_spmd(
        nc, in_maps, core_ids=list(range(NCORES)), trace=trace
    )
    # gather: outputs are feature-major [NB, E, S]; transpose back on host
    out_fm = np.concatenate([r_["out_fm"] for r_ in res.results], axis=0)
    outp = np.ascontiguousarray(out_fm.transpose(0, 2, 1))
    return outp, res


def kernel(**inputs) -> np.ndarray:
    # retry once on transient accelerator errors (rare NRT exec glitches)
    last = None
    for attempt in range(2):
        try:
            outp, _ = run(inputs, trace=False)
            return outp
        except Exception as e:  # noqa: BLE001
            last = e
            time.sleep(10)
    raise last


# revision 30
# speedup vs baseline: 1.2310x; 1.0169x over previous
"""Trainium2 Bass kernel for CLIPAttention (B=32, S=512, E=768, H=12, D=64).

Strategy: data-parallel over batch across 8 NeuronCores (4 batches/core).
All matmul operands fp16 (PSUM accumulates fp32). The kernel is PE-issue
bound, so the design minimizes tensor-engine instructions:

  - hidden_states is transposed to feature-major ON THE HOST and DMA'd
    straight into SBUF: no on-chip transposes at all.
  - q,k projections feature-major (lhsT = W tile, rhs = xt, N=512).
  - v token-major, written into v_ext[:, i, h, 0:64] with cols 64:128 of
    each head slot holding ones: the PV matmul lhsT = [v_h | ones] then
    computes BOTH the PV product (PSUM rows 0:64) and the softmax
    denominator replicated on rows 64:128 - the separate ones-matmul and
    the gpsimd partition-broadcast of the baseline are gone.
  - scores computed k-major (scoresT = kh.T @ qh) triangular; exp on ACT
    straight to fp16 SBUF; diagonal block masked by a 0/1 triu multiply.
  - out projection FEATURE-major (lhsT = Wo tile, rhs = outT), output is
    DMA'd feature-major and transposed back on the host.
  - emission is software-pipelined across batches: scores/PV of batch b
    interleave with the projections of batch b+1 so the PE never waits
    on the ACT exp chain.
"""

import os
import time

import numpy as np
from contextlib import ExitStack

import concourse.bass as bass
import concourse.mybir as mybir
import concourse.tile as tile
from concourse import bacc
from concourse.bass_utils import run_bass_kernel_spmd
from concourse.masks import make_upper_triangular

B, S, E, H, D = 32, 512, 768, 12, 64
NCORES = 8
NB = B // NCORES          # batches per core
P = 128
KT = E // P               # 6 feature tiles
QT = S // P               # 4 token tiles
SCALE = float(D) ** -0.5  # 0.125
F32 = mybir.dt.float32
F16 = mybir.dt.float16

AF = mybir.ActivationFunctionType
OP = mybir.AluOpType


def _build():
    nc = bacc.Bacc(trn_type="TRN2")

    hsT = nc.dram_tensor("hsT", [NB, E, S], F16, kind="ExternalInput")
    w_dr = {}
    b_dr = {}
    for nm in ("q", "k", "v", "o"):
        w_dr[nm] = nc.dram_tensor(f"W{nm}", [E, E], F16, kind="ExternalInput")
        b_dr[nm] = nc.dram_tensor(f"b{nm}", [E], F32, kind="ExternalInput")
    out = nc.dram_tensor("out_fm", [NB, E, S], F32, kind="ExternalOutput")

    with ExitStack() as ctx:
        tc = ctx.enter_context(tile.TileContext(nc))

        singles = ctx.enter_context(tc.tile_pool(name="singles", bufs=1))
        xtpool = ctx.enter_context(tc.tile_pool(name="xtpool", bufs=2))
        qkpool = ctx.enter_context(tc.tile_pool(name="qkpool", bufs=2))
        pepool = ctx.enter_context(tc.tile_pool(name="pepool", bufs=6))
        rpool = ctx.enter_context(tc.tile_pool(name="rpool", bufs=3))
        otpool = ctx.enter_context(tc.tile_pool(name="otpool", bufs=2))
        opool = ctx.enter_context(tc.tile_pool(name="opool", bufs=3))

        ps_mm = ctx.enter_context(tc.tile_pool(name="ps_mm", bufs=3, space="PSUM"))
        ps_s = ctx.enter_context(tc.tile_pool(name="ps_s", bufs=4, space="PSUM"))
        ps_pv = ctx.enter_context(tc.tile_pool(name="ps_pv", bufs=1, space="PSUM"))

        # ---- constants ----
        # upper-triangular (incl diagonal) 0/1 mask: keeps q >= k entries of
        # a k-major diagonal block
        triu01 = singles.tile([P, P], F16, name="triu01")
        make_upper_triangular(nc, triu01, val=1.0, diag=True)

        # weights (pre-cast fp16 on host); per-k-tile DMAs so the first
        # projection can start as soon as its tiles land, and the transfers
        # spread across DMA queues
        w_sb = {}
        for nm in ("q", "k", "v", "o"):
            w_sb[nm] = singles.tile([P, KT, E], F16, name=f"w_{nm}")
        xt_tiles = {}

        def dma_w(nm):
            nc.sync.dma_start(
                out=w_sb[nm], in_=w_dr[nm].rearrange("(ko p) m -> p ko m", p=P)
            )

        def dma_xt(b):
            nc.sync.dma_start(
                out=xt_tiles[b], in_=hsT[b].rearrange("(ko p) t -> p ko t", p=P)
            )

        for b in range(NB):
            xt_tiles[b] = xtpool.tile([P, KT, S], F16, name=f"xt_{b}", tag="xt")

        # bulk transfers in first-needed order; ONE dma_start per tensor
        # (descriptor issue on the sync engine is ~0.7us each, so fewer,
        # bigger issues start the data flowing sooner - the single striped
        # hardware queue gives full bandwidth either way)
        # first piece: just the m=0 column block of Wq plus batch-0 x, so
        # the very first projection chunk starts after ~1MB of DMA
        nc.sync.dma_start(
            out=w_sb["q"][:, :, :P],
            in_=w_dr["q"].rearrange("(ko p) m -> p ko m", p=P)[:, :, :P],
        )
        dma_xt(0)
        nc.sync.dma_start(
            out=w_sb["q"][:, :, P:],
            in_=w_dr["q"].rearrange("(ko p) m -> p ko m", p=P)[:, :, P:],
        )
        # biases are tiny but gate PSUM evacuation; right after the first two
        dma_w("k")
        bias_pp = {}
        for nm in ("q", "k", "o"):
            bias_pp[nm] = singles.tile([P, KT], F32, name=f"bpp_{nm}")
            nc.sync.dma_start(
                out=bias_pp[nm], in_=b_dr[nm].rearrange("(ko p) -> p ko", p=P)
            )
        # broadcast-to-all-partitions bias form for token-major v
        bias_bc = {}
        for nm in ("v",):
            bias_bc[nm] = singles.tile([P, E], F32, name=f"bbc_{nm}")
            src = b_dr[nm][:]
            bcast = bass.AP(tensor=src.tensor, offset=src.offset, ap=[[0, P], *src.ap])
            nc.sync.dma_start(out=bias_bc[nm], in_=bcast)
        dma_w("v")
        dma_xt(1)
        dma_w("o")
        dma_xt(2)
        dma_xt(3)

        # v_ext: [P, QT, H, 128] fp16; per head slot cols 0:64 hold ones,
        # cols 64:128 hold v_h (so the PV matmul also produces the softmax
        # denominator on PSUM partitions 0:64, where the custom-DVE
        # reciprocal can read it - it requires partition base 0). Two manual
        # buffers with the ones columns initialized once each.
        v_ext_bufs = []
        for i in range(2):
            vb = singles.tile([P, QT, H, P], F16, name=f"v_ext{i}")
            nc.vector.memset(vb[:, :, :, :D], 1.0)
            v_ext_bufs.append(vb)

        # ---------- per-batch state ----------
        qk_of = {}     # b -> {"q": tile, "k": tile}
        outT_of = {}   # b -> tile
        pE_live = {}   # (b,h) -> tile

        # ---------- chunk emitters ----------
        def qk_chunk(b, nm, m):
            if m == 0 and nm == "q":
                qk_of[b] = {}
            if nm not in qk_of[b]:
                qk_of[b][nm] = qkpool.tile(
                    [P, KT, S], F16, name=f"{nm}T_{b}", tag=f"{nm}T"
                )
            dst = qk_of[b][nm]
            xt = xt_tiles[b]
            ps = ps_mm.tile([P, S], F32, name=f"ps{nm}_{b}_{m}", tag="mm")
            for kk in range(KT):
                nc.tensor.matmul(
                    ps,
                    lhsT=w_sb[nm][:, kk, m * P:(m + 1) * P],
                    rhs=xt[:, kk, :],
                    start=(kk == 0),
                    stop=(kk == KT - 1),
                )
            if m % 2 == 0 and b < 3:
                nc.scalar.activation(
                    out=dst[:, m, :],
                    in_=ps,
                    func=AF.Identity,
                    bias=bias_pp[nm][:, m:m + 1],
                    scale=1.0,
                )
            else:
                nc.vector.tensor_scalar_add(
                    out=dst[:, m, :],
                    in0=ps,
                    scalar1=bias_pp[nm][:, m:m + 1],
                )

        def v_chunk(b, i):
            v_ext = v_ext_bufs[b % 2]
            xt = xt_tiles[b]
            # kk-outer so both N-chunks share one stationary load per k-tile
            pss = [
                ps_mm.tile([P, S], F32, name=f"psv_{b}_{i}_{n}", tag="mm")
                for n in range(2)
            ]
            for kk in range(KT):
                for n in range(2):
                    mm = nc.tensor.matmul(
                        pss[n][:, :384],
                        lhsT=xt[:, kk, i * P:(i + 1) * P],
                        rhs=w_sb["v"][:, kk, n * 384:(n + 1) * 384],
                        start=(kk == 0),
                        stop=(kk == KT - 1),
                    )
                    if n == 1:
                        mm.ldweights = False
            for n in range(2):
                nc.vector.tensor_tensor(
                    out=v_ext[:, i, n * 6:(n + 1) * 6, D:],
                    in0=pss[n][:, :384].rearrange("p (h c) -> p h c", c=D),
                    in1=bias_bc["v"][:, n * 384:(n + 1) * 384].rearrange(
                        "p (h c) -> p h c", c=D
                    ),
                    op=OP.add,
                )

        def sc_pair_chunk(b, h0, js):
            """Scores for heads h0 (even, PE rows 0:64 -> tile (0,0)) and
            h0+1 (odd, PE rows 64:128 -> tile (64,0)), j-interleaved so the
            two row-tiles of the 64x128-mode array compute concurrently."""
            heads = (h0, h0 + 1)
            if js[0] == 0:
                for h in heads:
                    pE_live[(b, h)] = pepool.tile(
                        [P, QT, S], F16, name=f"pE_{b}_{h}", tag="pE"
                    )
            pEs = {h: pE_live[(b, h)] for h in heads}
            for j in js:
                q0 = j * P
                n_mm = S - q0
                for h in heads:
                    g, rr = h // 2, h % 2
                    qh = qk_of[b]["q"][rr * D:(rr + 1) * D, g, :]
                    kh = qk_of[b]["k"][rr * D:(rr + 1) * D, g, :]
                    pE = pEs[h]
                    ps = ps_s.tile([P, S], F32, name=f"pss_{b}_{h}_{j}", tag="s")
                    nc.tensor.matmul(
                        ps[:, :n_mm],
                        lhsT=kh[:, j * P:(j + 1) * P],
                        rhs=qh[:, q0:],
                        start=True,
                        stop=True,
                    )
                    nc.scalar.activation(
                        out=pE[:, j, q0:],
                        in_=ps[:, :n_mm],
                        func=AF.Exp,
                        scale=SCALE,
                    )
                    # causal mask on the diagonal block: keep q >= k (on the
                    # otherwise-idle gpsimd engine, off the DVE critical path)
                    nc.gpsimd.tensor_tensor(
                        out=pE[:, j, q0:q0 + P],
                        in0=pE[:, j, q0:q0 + P],
                        in1=triu01,
                        op=OP.mult,
                    )

        def pv_chunk(b, h):
            g, rr = h // 2, h % 2
            if h == 0:
                outT_of[b] = otpool.tile(
                    [P, KT, S], F16, name=f"outT_{b}", tag="outT"
                )
            v_ext = v_ext_bufs[b % 2]
            pE = pE_live.pop((b, h))
            # PV + denominator in one matmul: lhsT = [v_h | ones]
            po = ps_pv.tile([P, S], F32, name=f"po_{b}_{h}", tag="pv")
            for j in range(QT):
                nc.tensor.matmul(
                    po[:, j * P:],
                    lhsT=v_ext[:, j, h, :],
                    rhs=pE[:, j, j * P:],
                    start=(j == 0),
                    stop=(j == QT - 1),
                    skip_group_check=True,
                )
            # reciprocal of the denominator (replicated on rows 0:64)
            rden = rpool.tile([D, S], F32, name=f"rden_{b}_{h}", tag="rden")
            nc.vector.reciprocal_approx_fast(rden, po[:D, :])
            # normalization fused into the PSUM->SBUF copy; partition-
            # shifted write puts odd heads at partitions 64:127 directly
            nc.vector.tensor_tensor(
                out=outT_of[b][rr * D:(rr + 1) * D, g, :],
                in0=po[D:, :],
                in1=rden,
                op=OP.mult,
            )

        def o_chunk(b, m):
            outT = outT_of[b]
            ps = ps_mm.tile([P, S], F32, name=f"pso_{b}_{m}", tag="mm")
            for kk in range(KT):
                nc.tensor.matmul(
                    ps,
                    lhsT=w_sb["o"][:, kk, m * P:(m + 1) * P],
                    rhs=outT[:, kk, :],
                    start=(kk == 0),
                    stop=(kk == KT - 1),
                )
            o_sb = opool.tile([P, S], F32, name=f"o_{b}_{m}", tag="o")
            if m % 2 == 0 and b < 2:
                nc.scalar.activation(
                    out=o_sb,
                    in_=ps,
                    func=AF.Identity,
                    bias=bias_pp["o"][:, m:m + 1],
                    scale=1.0,
                )
            else:
                nc.vector.tensor_scalar_add(
                    out=o_sb, in0=ps, scalar1=bias_pp["o"][:, m:m + 1]
                )
            nc.sync.dma_start(out=out[b, m * P:(m + 1) * P, :], in_=o_sb)

        # ---------- software-pipelined emission ----------
        def emit_slot(d_batch, w_chunks):
            """Interleave scores/pv of batch d_batch with the given list of
            projection chunk thunks (next batch's q/k/v, prior batch's o)."""
            if d_batch is None:
                for w in w_chunks:
                    w()
                return
            wi = 0

            def take_w(k):
                nonlocal wi
                for _ in range(k):
                    if wi < len(w_chunks):
                        w_chunks[wi]()
                        wi += 1

            for p in range(H // 2):
                sc_pair_chunk(d_batch, 2 * p, (0, 1))
                if p >= 1:
                    pv_chunk(d_batch, 2 * p - 2)
                take_w(1)
                sc_pair_chunk(d_batch, 2 * p, (2, 3))
                take_w(1)
                if p >= 1:
                    pv_chunk(d_batch, 2 * p - 1)
                take_w(1)
            pv_chunk(d_batch, H - 2)
            take_w(2)
            pv_chunk(d_batch, H - 1)
            take_w(len(w_chunks) - wi)

        def qk_chunks(b):
            return [
                (lambda b=b, nm=nm, m=m: qk_chunk(b, nm, m))
                for nm in ("q", "k")
                for m in range(KT)
            ]

        def v_chunks(b):
            return [(lambda b=b, i=i: v_chunk(b, i)) for i in range(QT)]

        def o_chunks(b):
            return [(lambda b=b, m=m: o_chunk(b, m)) for m in range(KT)]

        # slot 0: projections of batch 0 only
        emit_slot(None, qk_chunks(0) + v_chunks(0))
        # steady-state slots; v(3) runs at the head of slot 4 so the thin
        # tail slot still has PE work to hide the batch-3 exp chain
        emit_slot(0, qk_chunks(1) + v_chunks(1))
        emit_slot(1, qk_chunks(2) + v_chunks(2) + o_chunks(0))
        emit_slot(2, qk_chunks(3) + o_chunks(1))
        emit_slot(3, v_chunks(3) + o_chunks(2))
        emit_slot(None, o_chunks(3))

    nc.compile()
    return nc


_NC_CACHE = None


def _get_nc():
    global _NC_CACHE
    if _NC_CACHE is None:
        _NC_CACHE = _build()
    return _NC_CACHE


def run(inputs, trace=False):
    if trace:
        os.environ.pop("BASS_NEVER_TRACE", None)
    else:
        # keep the spmd runner off the NTFF trace path (the profiling hook
        # module is not always present)
        os.environ["BASS_NEVER_TRACE"] = "1"
    # host prep: cast to fp16 and pre-transpose hidden_states to
    # feature-major (identical rounding to an on-chip cast, and it removes
    # every on-chip transpose)
    hs = np.asarray(inputs["hidden_states"], dtype=np.float32).astype(np.float16)
    assert hs.shape == (B, S, E)
    hsT = np.ascontiguousarray(hs.transpose(0, 2, 1))  # [B, E, S]
    wb = {}
    for nm in ("q", "k", "v", "o"):
        wb[f"W{nm}"] = np.ascontiguousarray(
            np.asarray(inputs[f"W{nm}"], dtype=np.float32).astype(np.float16)
        )
        wb[f"b{nm}"] = np.ascontiguousarray(
            np.asarray(inputs[f"b{nm}"], dtype=np.float32)
        )

    nc = _get_nc()
    in_maps = []
    for c in range(NCORES):
        m = {"hsT": hsT[c * NB:(c + 1) * NB]}
        m.update(wb)
        in_maps.append(m)
    res = run_bass_kernel_spmd(
        nc, in_maps, core_ids=list(range(NCORES)), trace=trace
    )
    # gather: outputs are feature-major [NB, E, S]; transpose back on host
    out_fm = np.concatenate([r_["out_fm"] for r_ in res.results], axis=0)
    outp = np.ascontiguousarray(out_fm.transpose(0, 2, 1))
    return outp, res


def kernel(**inputs) -> np.ndarray:
    # retry once on transient accelerator errors (rare NRT exec glitches)
    last = None
    for attempt in range(2):
        try:
            outp, _ = run(inputs, trace=False)
            return outp
        except Exception as e:  # noqa: BLE001
            last = e
            time.sleep(10)
    raise last
